# revision 7
# baseline (speedup 1.0000x reference)
"""BitLinear forward on 8 TRN2 NeuronCores — data-parallel over tokens.

Math: reference computes
    gamma_w = mean|W| + eps;  bw = clip(round(W/gamma_w), -1, 1)
    xn = LayerNorm(x);  gamma = max|xn|;  xq = clip(xn*QB/gamma, +-(QB-eps))
    y  = (xq @ bw.T) * (gamma*beta/QB),  beta = max_d sum_o |W[o,d]|
The gamma factor cancels algebraically (clip only nudges the max element
by 1e-5/127 ~ 8e-8 relative), so on device we compute
    y = (LayerNorm(x) @ bw.T) * beta
with NO cross-core collective.  Ternary weights use the sign LUT
    stored = sign(W-thr) + (-2)*[W<=-thr] = 2*bw - 1 in {1,-1,-3}
(thr = gamma_w/2); the uniform -1 offset and the factor 2 cancel through
the per-token epilogue
    y[t,o] = rstd[t]*(beta/2) * (ps[t,o] - mu[t]*csm1[o])
with ps = x @ stored and csm1[o] = sum_d stored[d,o] (~ -2048, kept in
f32).  The rank-1 -mu*csm1 correction runs on the (otherwise idle)
Vector engine during the main-matmul phase instead of costing TensorE
cycles like the baseline's extra accumulation matmul.

Schedule (the 424us baseline spent 0-190us on ingest/stats/ternarize
with every engine at ~50%, then 190-410us in a perfect PE-bound matmul):
 - stored ternary weights are FP8E4 (exact for {1,-1,-3}); the main
   matmul runs bf16 stationary x fp8 moving at full rate (verified on
   HW: error identical to bf16xbf16).  Their SBUF footprint halves, so
   5 of the 16 f32 W tiles survive the stats pass in SBUF and only 11
   are re-read for ternarization (the baseline re-read all 16).
 - x is read ONCE in 512-token slabs (2KB DMA lines), converted to
   bf16 into per-(k,slab) resident tiles that serve both the LN-stats
   matmuls and the main matmul.
 - program order (= DMA queue order = per-engine FIFO order) is
   interleaved so the main matmul starts ~90us in, chasing the
   ternarize pipeline, with later x slabs + their stats woven between
   m-blocks; y is written per-m as bf16 (halves write traffic, +0.1%
   rms on values ~39 in magnitude), upcast to f32 on host.
"""

import os
import sys

import numpy as np

for _p in ("/opt/trn_rl_repo", "/root/.axon_site/_ro/trn_rl_repo"):
    if os.path.isdir(_p) and _p not in sys.path:
        sys.path.append(_p)

from concourse import bacc, bass_isa, mybir, tile  # noqa: E402
from concourse.bass_utils import run_bass_kernel_spmd  # noqa: E402

P = 128
D = 2048  # contraction (hidden) dim
O = 2048  # output dim
N_CORES = 8
N_TOK = 4 * 4096
TOK = N_TOK // N_CORES  # tokens per core
KT = D // P  # 16 contraction tiles
MT = TOK // P  # 16 token tiles per core
CH = 512  # psum free chunk (one bank of f32)
NCH = O // CH
NS = TOK // CH  # 4 x slabs of 512 tokens
MPS = MT // NS  # 4 m-blocks per slab
WK = 5  # W f32 tiles kept resident after the stats pass
EPS = 1e-5
F32 = mybir.dt.float32
BF16 = mybir.dt.bfloat16
FP8 = mybir.dt.float8e4


def build_nc():
    nc = bacc.Bacc(None, target_bir_lowering=False, debug=False)
    xt = nc.declare_dram_parameter("xt", [D, TOK], F32, isOutput=False)
    fwt = nc.declare_dram_parameter("fwt", [D, O], F32, isOutput=False)
    y = nc.declare_dram_parameter("y", [TOK, O], BF16, isOutput=True)

    Alu = mybir.AluOpType
    Act = mybir.ActivationFunctionType
    Ax = mybir.AxisListType

    with tile.TileContext(nc) as tc:
        with (
            tc.tile_pool(name="const", bufs=1) as const,
            tc.tile_pool(name="wkeep", bufs=1) as wkeep,
            tc.tile_pool(name="wtmp", bufs=2) as wtmp,
            tc.tile_pool(name="bwb", bufs=2) as bwbp,
            tc.tile_pool(name="bneg", bufs=2) as bnegp,
            tc.tile_pool(name="bw8", bufs=1) as bw8p,
            tc.tile_pool(name="xbs", bufs=1) as xbsp,
            tc.tile_pool(name="x1", bufs=2) as x1p,
            tc.tile_pool(name="xsq", bufs=2) as xsqp,
            tc.tile_pool(name="cs", bufs=1) as csp,
            tc.tile_pool(name="stt", bufs=1) as stt,
            tc.tile_pool(name="ep", bufs=2) as epp,
            tc.tile_pool(name="ysb", bufs=2) as ysbp,
            tc.tile_pool(name="yout", bufs=2) as youtp,
            tc.tile_pool(name="dram", bufs=1, space="DRAM") as dpool,
            tc.tile_pool(name="paux", bufs=1, space="PSUM") as paux,
            tc.tile_pool(name="pmm", bufs=1, space="PSUM") as pmm,
        ):
            ones_b = const.tile([P, P], BF16)
            nc.vector.memset(ones_b, 1.0)
            eps_t = const.tile([P, 1], F32)
            nc.vector.memset(eps_t, EPS)
            scal = const.tile([P, 8], F32)  # on-device scalar registry
            wsum = const.tile([P, KT], F32)  # per-partition |W| row sums
            mucol = const.tile([P, MT], F32)  # mu[t] columnized
            acol = const.tile([P, MT], F32)  # rstd[t]*beta/2 columnized
            ccol = const.tile([P, MT], F32)  # acol*mu columnized
            rbcol = const.tile([P, MT], F32)
            mu_d = dpool.tile([TOK], F32)
            rb_d = dpool.tile([TOK], F32)
            mu_g = mu_d.rearrange("(m p) -> p m", p=P)
            rb_g = rb_d.rearrange("(m p) -> p m", p=P)

            # resident bf16 x tiles, one per (k, slab)
            xbt = {}

            def x_ingest(s, k):
                x1 = x1p.tile([P, CH], F32, tag="x1")
                nc.sync.dma_start(
                    x1, xt[P * k : P * (k + 1), CH * s : CH * (s + 1)]
                )
                xb = xbsp.tile([P, CH], BF16, tag=f"xb{k}_{s}", name=f"xb{k}_{s}")
                nc.vector.tensor_copy(out=xb, in_=x1)
                xbt[(k, s)] = xb

            ps_mu = {}
            ps_sq = {}

            def stats_mm(s, k):
                # mu/sumsq contributions of slab (k, s) via ones-matmul
                xb = xbt[(k, s)]
                xq = xsqp.tile([P, CH], BF16, tag="xsq")
                nc.vector.tensor_tensor(out=xq, in0=xb, in1=xb, op=Alu.mult)
                if k == 0:
                    ps_mu[s] = paux.tile([P, CH], F32, tag="a0", name=f"pmu{s}")
                    ps_sq[s] = paux.tile([P, CH], F32, tag="a1", name=f"psq{s}")
                nc.tensor.matmul(
                    ps_mu[s], ones_b, xb, start=(k == 0), stop=(k == KT - 1)
                )
                nc.tensor.matmul(
                    ps_sq[s], ones_b, xq, start=(k == 0), stop=(k == KT - 1)
                )

            def stats_fin(s):
                # finalize LN stats for slab s on partition row 0 only
                sl = slice(CH * s, CH * (s + 1))
                msl = slice(MPS * s, MPS * (s + 1))
                mu_c = stt.tile([1, CH], F32, tag="fmu")
                nc.vector.tensor_scalar(
                    out=mu_c, in0=ps_mu[s][0:1, :], scalar1=1.0 / D,
                    scalar2=None, op0=Alu.mult,
                )
                ex2 = stt.tile([1, CH], F32, tag="fex")
                nc.vector.tensor_scalar(
                    out=ex2, in0=ps_sq[s][0:1, :], scalar1=1.0 / D,
                    scalar2=None, op0=Alu.mult,
                )
                musq = stt.tile([1, CH], F32, tag="fmsq")
                nc.vector.tensor_tensor(out=musq, in0=mu_c, in1=mu_c, op=Alu.mult)
                nc.vector.tensor_tensor(out=ex2, in0=ex2, in1=musq, op=Alu.subtract)
                nc.scalar.activation(ex2, ex2, Act.Sqrt, bias=eps_t[0:1, :])
                rstd = stt.tile([1, CH], F32, tag="frst")
                nc.vector.reciprocal(rstd, ex2)
                # columnize via DRAM gather round-trip
                nc.sync.dma_start(mu_d[None, sl], mu_c)
                nc.sync.dma_start(rb_d[None, sl], rstd)
                with nc.allow_non_contiguous_dma(reason="512x4B stat gather"):
                    nc.sync.dma_start(mucol[:, msl], mu_g[:, msl])
                    nc.sync.dma_start(rbcol[:, msl], rb_g[:, msl])
                # acol = rstd*beta/2 ; ccol = acol*mu
                nc.vector.tensor_scalar(
                    out=acol[:, msl], in0=rbcol[:, msl],
                    scalar1=scal[:, 6:7], scalar2=None, op0=Alu.mult,
                )
                nc.vector.tensor_tensor(
                    out=ccol[:, msl], in0=acol[:, msl], in1=mucol[:, msl],
                    op=Alu.mult,
                )

            # ---- W stats pass (Wa) interleaved with x slab s=0 ----------
            wres = {}
            for k in range(KT):
                if k < WK:
                    wt = wkeep.tile([P, O], F32, tag=f"wk{k}", name=f"wk{k}")
                    wres[k] = wt
                else:
                    wt = wtmp.tile([P, O], F32, tag="wt")
                nc.sync.dma_start(wt, fwt[P * k : P * (k + 1), :])
                abs_t = bwbp.tile([P, O], BF16, tag="bwb")
                nc.scalar.activation(
                    abs_t, wt, Act.Abs, accum_out=wsum[:, k : k + 1]
                )
                x_ingest(0, k)
                stats_mm(0, k)

            # ---- W stat reduce: thr = gamma_w/2, beta -------------------
            row_tot = scal[:, 0:1]
            nc.vector.tensor_reduce(row_tot, wsum, axis=Ax.X, op=Alu.add)
            beta_pp = scal[:, 1:2]
            nc.vector.tensor_reduce(beta_pp, wsum, axis=Ax.X, op=Alu.max)
            tot_b = scal[:, 2:3]
            nc.gpsimd.partition_all_reduce(
                tot_b, row_tot, channels=P, reduce_op=bass_isa.ReduceOp.add
            )
            beta_b = scal[:, 3:4]
            nc.gpsimd.partition_all_reduce(
                beta_b, beta_pp, channels=P, reduce_op=bass_isa.ReduceOp.max
            )
            thr = scal[:, 4:5]
            nc.scalar.activation(
                thr, tot_b, Act.Copy, bias=0.5 * EPS, scale=0.5 / (D * O)
            )
            nthr = scal[:, 5:6]
            nc.scalar.activation(
                nthr, tot_b, Act.Copy, bias=-0.5 * EPS, scale=-0.5 / (D * O)
            )
            beta_h = scal[:, 6:7]  # beta/2 (stored weights carry factor 2)
            nc.scalar.activation(beta_h, beta_b, Act.Copy, bias=0.0, scale=0.5)

            stats_fin(0)

            # ---- re-read the non-resident W tiles -----------------------
            wre = {}
            for k in range(WK, KT):
                wt = wtmp.tile([P, O], F32, tag="wt")
                nc.sync.dma_start(wt, fwt[P * k : P * (k + 1), :])
                wre[k] = wt

            # ---- ternarize, chased on PE by colsum + m0 matmuls ---------
            cs_f = csp.tile([P, O], F32)  # csm1 broadcast, exact f32
            ps_cs = [
                paux.tile([P, CH], F32, tag=f"a{c}", name=f"pcs{c}")
                for c in range(NCH)
            ]
            pys0 = [
                pmm.tile([P, CH], F32, tag=f"mm{c}", name=f"py0_{c}")
                for c in range(NCH)
            ]
            bw8s = []
            for k in range(KT):
                src = wres[k] if k < WK else wre[k]
                bwb = bwbp.tile([P, O], BF16, tag="bwb")
                nc.scalar.activation(bwb, src, Act.Sign, bias=nthr)
                bneg = bnegp.tile([P, O], BF16, tag="bneg")
                nc.vector.tensor_scalar(
                    out=bneg, in0=src, scalar1=nthr, scalar2=-2.0,
                    op0=Alu.is_le, op1=Alu.mult,
                )
                bw8 = bw8p.tile([P, O], FP8, tag=f"bw{k}", name=f"bw{k}")
                nc.vector.tensor_tensor(out=bw8, in0=bwb, in1=bneg, op=Alu.add)
                bw8s.append(bw8)
                for c in range(NCH):
                    sl = slice(CH * c, CH * (c + 1))
                    nc.tensor.matmul(
                        ps_cs[c], ones_b, bw8[:, sl],
                        start=(k == 0), stop=(k == KT - 1),
                    )
                    nc.tensor.matmul(
                        pys0[c], xbt[(k, 0)][:, 0:P], bw8[:, sl],
                        start=(k == 0), stop=(k == KT - 1),
                    )
            for c in range(NCH):
                nc.scalar.copy(cs_f[:, CH * c : CH * (c + 1)], ps_cs[c])

            # ---- main matmul + epilogue ---------------------------------
            def epilogue(m, pys):
                yo = youtp.tile([P, O], BF16, tag="yo")
                for c in range(NCH):
                    sl = slice(CH * c, CH * (c + 1))
                    ysb = ysbp.tile([P, CH], F32, tag="ysb")
                    nc.scalar.mul(ysb, pys[c], acol[:, m : m + 1])
                    tmp = epp.tile([P, CH], F32, tag="ep")
                    nc.vector.tensor_scalar(
                        out=tmp, in0=cs_f[:, sl], scalar1=ccol[:, m : m + 1],
                        scalar2=None, op0=Alu.mult,
                    )
                    nc.vector.tensor_tensor(
                        out=yo[:, sl], in0=ysb, in1=tmp, op=Alu.subtract
                    )
                nc.sync.dma_start(y[P * m : P * (m + 1), :], yo)

            def main_mm(m):
                if m == 0:
                    pys = pys0
                else:
                    pys = [
                        pmm.tile([P, CH], F32, tag=f"mm{c}", name=f"py{m}_{c}")
                        for c in range(NCH)
                    ]
                    s, j = m // MPS, m % MPS
                    for k in range(KT):
                        lhs = xbt[(k, s)][:, P * j : P * (j + 1)]
                        for c in range(NCH):
                            nc.tensor.matmul(
                                pys[c], lhs, bw8s[k][:, CH * c : CH * (c + 1)],
                                start=(k == 0), stop=(k == KT - 1),
                            )
                epilogue(m, pys)

            # interleave remaining x slabs (ingest early; their stats
            # matmuls late so the PE never waits on them) between m-groups.
            for k in range(KT):
                x_ingest(1, k)
            epilogue(0, pys0)
            main_mm(1)
            for k in range(KT):
                stats_mm(1, k)
            stats_fin(1)
            main_mm(2)
            main_mm(3)
            for k in range(KT):
                x_ingest(2, k)
            main_mm(4)
            main_mm(5)
            for k in range(KT):
                stats_mm(2, k)
            stats_fin(2)
            main_mm(6)
            for k in range(KT):
                x_ingest(3, k)
            main_mm(7)
            main_mm(8)
            main_mm(9)
            for k in range(KT):
                stats_mm(3, k)
            stats_fin(3)
            for m in range(10, MT):
                main_mm(m)

    nc.compile()
    return nc


_NC_CACHE = None


def _get_nc():
    global _NC_CACHE
    if _NC_CACHE is None:
        _NC_CACHE = build_nc()
    return _NC_CACHE


def _prep_in_maps(x, fweight):
    x2 = np.ascontiguousarray(x, dtype=np.float32).reshape(N_TOK, D)
    fwt = np.ascontiguousarray(np.asarray(fweight, dtype=np.float32).T)
    in_maps = []
    for c in range(N_CORES):
        xs = np.ascontiguousarray(x2[c * TOK : (c + 1) * TOK, :].T)
        in_maps.append({"xt": xs, "fwt": fwt})
    return in_maps


def run_spmd(x, fweight, **kw):
    nc = _get_nc()
    in_maps = _prep_in_maps(x, fweight)
    return run_bass_kernel_spmd(nc, in_maps, core_ids=list(range(N_CORES)), **kw)


def kernel(x, fweight):
    res = run_spmd(x, fweight)
    y = np.concatenate(
        [np.asarray(res.results[c]["y"], dtype=np.float32) for c in range(N_CORES)],
        axis=0,
    )
    return y.reshape(4, 4096, O)


if __name__ == "__main__":
    xx = np.random.randn(4, 4096, D).astype(np.float32)
    ww = np.random.uniform(-1 / np.sqrt(D), 1 / np.sqrt(D), (O, D)).astype(np.float32)
    out = kernel(xx, ww)
    print("out", out.shape, out.dtype, float(np.abs(out).mean()))


# revision 9
# speedup vs baseline: 1.0846x; 1.0846x over previous
"""BitLinear forward on 8 TRN2 NeuronCores — data-parallel over tokens.

Math: reference computes
    gamma_w = mean|W| + eps;  bw = clip(round(W/gamma_w), -1, 1)
    xn = LayerNorm(x);  gamma = max|xn|;  xq = clip(xn*QB/gamma, +-(QB-eps))
    y  = (xq @ bw.T) * (gamma*beta/QB),  beta = max_d sum_o |W[o,d]|
The gamma factor cancels algebraically (clip only nudges the max element
by 1e-5/127 ~ 8e-8 relative), so on device we compute
    y = (LayerNorm(x) @ bw.T) * beta
with NO cross-core collective.  Ternary weights use the sign LUT
    stored = sign(W-thr) + (-2)*[W<=-thr] = 2*bw - 1 in {1,-1,-3}
(thr = gamma_w/2); the uniform -1 offset and the factor 2 cancel through
the per-token epilogue
    y[t,o] = rstd[t]*(beta/2) * (ps[t,o] - mu[t]*csm1[o])
with ps = x @ stored and csm1[o] = sum_d stored[d,o] (~ -2048, kept in
f32).  The rank-1 -mu*csm1 correction runs on the (otherwise idle)
Vector engine during the main-matmul phase (one tensor_scalar + one
in-place subtract per m-block) instead of costing TensorE cycles.

Key scheduling ideas (the 424us baseline spent 0-190us on
ingest/stats/ternarize with every engine at ~50%, then 190-410us in a
perfect PE-bound matmul):
 - stored ternary weights are FP8E4 (exact for {1,-1,-3}); the main
   matmul runs bf16 stationary x fp8 moving at full rate (verified on
   HW: error identical to bf16xbf16), halving their SBUF footprint.
 - 10 of the 16 f32 W tiles survive the stats pass in SBUF: 4 in a
   dedicated pool and 6 "borrowed" inside the (still empty) x-slab
   tiles for token slabs 1-3 via dtype-bitcast views — those slabs are
   only converted after ternarization finishes.  Only 6 W tiles are
   re-read from HBM (the baseline re-read all 16).
 - x is read ONCE in 512-token slabs (2KB DMA lines, two k-tiles per
   DMA), converted to bf16 into per-slab resident tiles serving both
   the LN-stats matmuls and the main matmul.
 - one PSUM pool with per-chunk tags and bufs=2 double-buffers
   everything (stats accum, colsum accum, m-group accum) in 8 banks.
 - program order (= DMA queue order = per-engine FIFO order) lets the
   colsum + m0 matmuls chase the ternarize pipeline, with later x
   slabs + their stats matmuls woven between m-groups; y is written
   per-m as bf16 (halves write traffic; +0.1% rms), upcast on host.
"""

import os
import sys

import numpy as np

for _p in ("/opt/trn_rl_repo", "/root/.axon_site/_ro/trn_rl_repo"):
    if os.path.isdir(_p) and _p not in sys.path:
        sys.path.append(_p)

from concourse import bacc, bass_isa, mybir, tile  # noqa: E402
from concourse.bass_utils import run_bass_kernel_spmd  # noqa: E402

P = 128
D = 2048  # contraction (hidden) dim
O = 2048  # output dim
N_CORES = 8
N_TOK = 4 * 4096
TOK = N_TOK // N_CORES  # tokens per core
KT = D // P  # 16 contraction tiles
MT = TOK // P  # 16 token tiles per core
CH = 512  # psum free chunk (one bank of f32)
NCH = O // CH
NS = TOK // CH  # 4 x slabs of 512 tokens
MPS = MT // NS  # 4 m-blocks per slab
WK = 4  # W f32 tiles in the dedicated resident pool
NB = 6  # W f32 tiles borrowed inside x-slab tiles (2 per slab 1..3)
EPS = 1e-5
F32 = mybir.dt.float32
BF16 = mybir.dt.bfloat16
FP8 = mybir.dt.float8e4


def build_nc():
    nc = bacc.Bacc(None, target_bir_lowering=False, debug=False)
    xt = nc.declare_dram_parameter("xt", [D, TOK], F32, isOutput=False)
    fwt = nc.declare_dram_parameter("fwt", [D, O], F32, isOutput=False)
    y = nc.declare_dram_parameter("y", [TOK, O], BF16, isOutput=True)

    Alu = mybir.AluOpType
    Act = mybir.ActivationFunctionType
    Ax = mybir.AxisListType

    with tile.TileContext(nc) as tc:
        with (
            tc.tile_pool(name="const", bufs=1) as const,
            tc.tile_pool(name="wkeep", bufs=1) as wkeep,
            tc.tile_pool(name="wtmp", bufs=2) as wtmp,
            tc.tile_pool(name="bwb", bufs=2) as bwbp,
            tc.tile_pool(name="bneg", bufs=2) as bnegp,
            tc.tile_pool(name="bw8", bufs=1) as bw8p,
            tc.tile_pool(name="xbig", bufs=1) as xbigp,
            tc.tile_pool(name="x1", bufs=2) as x1p,
            tc.tile_pool(name="xq", bufs=1) as xqp,
            tc.tile_pool(name="cs", bufs=1) as csp,
            tc.tile_pool(name="stt", bufs=1) as stt,
            tc.tile_pool(name="ep", bufs=1) as epp,
            tc.tile_pool(name="yout", bufs=2) as youtp,
            tc.tile_pool(name="dram", bufs=1, space="DRAM") as dpool,
            tc.tile_pool(name="ps8", bufs=2, space="PSUM") as ps8,
        ):
            ones_b = const.tile([P, P], BF16)
            nc.vector.memset(ones_b, 1.0)
            eps_t = const.tile([P, 1], F32)
            nc.vector.memset(eps_t, EPS)
            scal = const.tile([P, 8], F32)  # on-device scalar registry
            wsum = const.tile([P, KT], F32)  # per-partition |W| row sums
            mucol = const.tile([P, MT], F32)  # mu[t] columnized
            acol = const.tile([P, MT], F32)  # rstd[t]*beta/2 columnized
            ccol = const.tile([P, MT], F32)  # acol*mu columnized
            rbcol = const.tile([P, MT], F32)
            mu_d = dpool.tile([TOK], F32)
            rb_d = dpool.tile([TOK], F32)
            mu_g = mu_d.rearrange("(m p) -> p m", p=P)
            rb_g = rb_d.rearrange("(m p) -> p m", p=P)

            # resident bf16 x tiles, one per slab; slabs 1-3 double as f32
            # storage for 2 borrowed W tiles each until ternarization.
            xbig = [
                xbigp.tile([P, KT * CH], BF16, tag=f"xbig{s}", name=f"xbig{s}")
                for s in range(NS)
            ]

            def wsrc(k):
                # f32 source AP for W tile k during pass A / ternarize
                if k < WK:
                    return wres[k]
                if k < WK + NB:
                    j = k - WK
                    host = xbig[1 + j // 2][:, :].bitcast(F32)
                    return host[:, O * (j % 2) : O * (j % 2 + 1)]
                return wre[k]

            def x_ingest(s, kp):
                # one DMA + one convert for k-tiles (2kp, 2kp+1) of slab s
                x1 = x1p.tile([P, 2 * CH], F32, tag="x1")
                src = xt[2 * P * kp : 2 * P * (kp + 1),
                         CH * s : CH * (s + 1)]
                nc.sync.dma_start(
                    x1.rearrange("p (two t) -> p two t", two=2),
                    src.rearrange("(two p) t -> p two t", p=P),
                )
                nc.vector.tensor_copy(
                    out=xbig[s][:, 2 * CH * kp : 2 * CH * (kp + 1)], in_=x1
                )

            ps_mu = {}
            ps_sq = {}

            def stats_mm(s, g):
                # stats matmuls for k-group g (4 k-tiles) of slab s
                xq = xqp.tile([P, 4 * CH], BF16, tag="xq")
                gsl = slice(4 * CH * g, 4 * CH * (g + 1))
                nc.vector.tensor_tensor(
                    out=xq, in0=xbig[s][:, gsl], in1=xbig[s][:, gsl], op=Alu.mult
                )
                if g == 0:
                    ps_mu[s] = ps8.tile([P, CH], F32, tag="a0", name=f"pmu{s}")
                    ps_sq[s] = ps8.tile([P, CH], F32, tag="a1", name=f"psq{s}")
                for i in range(4):
                    k = 4 * g + i
                    nc.tensor.matmul(
                        ps_mu[s], ones_b, xbig[s][:, CH * k : CH * (k + 1)],
                        start=(k == 0), stop=(k == KT - 1),
                    )
                    nc.tensor.matmul(
                        ps_sq[s], ones_b, xq[:, CH * i : CH * (i + 1)],
                        start=(k == 0), stop=(k == KT - 1),
                    )

            def stats_fin(s):
                # finalize LN stats for slab s on partition row 0 only
                sl = slice(CH * s, CH * (s + 1))
                msl = slice(MPS * s, MPS * (s + 1))
                mu_c = stt.tile([1, CH], F32, tag="fmu")
                nc.vector.tensor_scalar(
                    out=mu_c, in0=ps_mu[s][0:1, :], scalar1=1.0 / D,
                    scalar2=None, op0=Alu.mult,
                )
                ex2 = stt.tile([1, CH], F32, tag="fex")
                nc.vector.tensor_scalar(
                    out=ex2, in0=ps_sq[s][0:1, :], scalar1=1.0 / D,
                    scalar2=None, op0=Alu.mult,
                )
                musq = stt.tile([1, CH], F32, tag="fmsq")
                nc.vector.tensor_tensor(out=musq, in0=mu_c, in1=mu_c, op=Alu.mult)
                nc.vector.tensor_tensor(out=ex2, in0=ex2, in1=musq, op=Alu.subtract)
                nc.scalar.activation(ex2, ex2, Act.Sqrt, bias=eps_t[0:1, :])
                rstd = stt.tile([1, CH], F32, tag="frst")
                nc.vector.reciprocal(rstd, ex2)
                # columnize via DRAM gather round-trip
                nc.sync.dma_start(mu_d[None, sl], mu_c)
                nc.sync.dma_start(rb_d[None, sl], rstd)
                with nc.allow_non_contiguous_dma(reason="512x4B stat gather"):
                    nc.sync.dma_start(mucol[:, msl], mu_g[:, msl])
                    nc.sync.dma_start(rbcol[:, msl], rb_g[:, msl])
                # acol = rstd*beta/2 ; ccol = acol*mu
                nc.vector.tensor_scalar(
                    out=acol[:, msl], in0=rbcol[:, msl],
                    scalar1=scal[:, 6:7], scalar2=None, op0=Alu.mult,
                )
                nc.vector.tensor_tensor(
                    out=ccol[:, msl], in0=acol[:, msl], in1=mucol[:, msl],
                    op=Alu.mult,
                )

            # ---- W stats pass (Wa) interleaved with x slab s=0 ----------
            wres = {}
            wre = {}
            for k in range(KT):
                if k < WK:
                    wt = wkeep.tile([P, O], F32, tag=f"wk{k}", name=f"wk{k}")
                    wres[k] = wt
                    dst = wt
                elif k < WK + NB:
                    dst = wsrc(k)
                else:
                    wt = wtmp.tile([P, O], F32, tag="wt")
                    dst = wt
                nc.sync.dma_start(dst, fwt[P * k : P * (k + 1), :])
                abs_t = bwbp.tile([P, O], BF16, tag="bwb")
                nc.scalar.activation(
                    abs_t, dst, Act.Abs, accum_out=wsum[:, k : k + 1]
                )
                if k % 2 == 1:
                    x_ingest(0, k // 2)
                if k % 4 == 3:
                    stats_mm(0, k // 4)

            # ---- W stat reduce: thr = gamma_w/2, beta -------------------
            row_tot = scal[:, 0:1]
            nc.vector.tensor_reduce(row_tot, wsum, axis=Ax.X, op=Alu.add)
            beta_pp = scal[:, 1:2]
            nc.vector.tensor_reduce(beta_pp, wsum, axis=Ax.X, op=Alu.max)
            tot_b = scal[:, 2:3]
            nc.gpsimd.partition_all_reduce(
                tot_b, row_tot, channels=P, reduce_op=bass_isa.ReduceOp.add
            )
            beta_b = scal[:, 3:4]
            nc.gpsimd.partition_all_reduce(
                beta_b, beta_pp, channels=P, reduce_op=bass_isa.ReduceOp.max
            )
            thr = scal[:, 4:5]
            nc.scalar.activation(
                thr, tot_b, Act.Copy, bias=0.5 * EPS, scale=0.5 / (D * O)
            )
            nthr = scal[:, 5:6]
            nc.scalar.activation(
                nthr, tot_b, Act.Copy, bias=-0.5 * EPS, scale=-0.5 / (D * O)
            )
            beta_h = scal[:, 6:7]  # beta/2 (stored weights carry factor 2)
            nc.scalar.activation(beta_h, beta_b, Act.Copy, bias=0.0, scale=0.5)

            stats_fin(0)

            # ---- re-read the 6 non-resident W tiles ---------------------
            for k in range(WK + NB, KT):
                wt = wtmp.tile([P, O], F32, tag="wt")
                nc.sync.dma_start(wt, fwt[P * k : P * (k + 1), :])
                wre[k] = wt

            # ---- ternarize, chased on PE by colsum + m0 matmuls ---------
            cs_f = csp.tile([P, O], F32)  # csm1 broadcast, exact f32
            ps_cs = [
                ps8.tile([P, CH], F32, tag=f"a{c}", name=f"pcs{c}")
                for c in range(NCH)
            ]
            pys0 = [
                ps8.tile([P, CH], F32, tag=f"a{c}", name=f"py0_{c}")
                for c in range(NCH)
            ]
            bw8s = []
            for k in range(KT):
                src = wsrc(k)
                bwb = bwbp.tile([P, O], BF16, tag="bwb")
                nc.scalar.activation(bwb, src, Act.Sign, bias=nthr)
                bneg = bnegp.tile([P, O], BF16, tag="bneg")
                nc.vector.tensor_scalar(
                    out=bneg, in0=src, scalar1=nthr, scalar2=-2.0,
                    op0=Alu.is_le, op1=Alu.mult,
                )
                bw8 = bw8p.tile([P, O], FP8, tag=f"bw{k}", name=f"bw{k}")
                nc.vector.tensor_tensor(out=bw8, in0=bwb, in1=bneg, op=Alu.add)
                bw8s.append(bw8)
                for c in range(NCH):
                    sl = slice(CH * c, CH * (c + 1))
                    nc.tensor.matmul(
                        ps_cs[c], ones_b, bw8[:, sl],
                        start=(k == 0), stop=(k == KT - 1),
                    )
                    nc.tensor.matmul(
                        pys0[c], xbig[0][:, CH * k : CH * k + P], bw8[:, sl],
                        start=(k == 0), stop=(k == KT - 1),
                    )
            for c in range(NCH):
                nc.scalar.copy(cs_f[:, CH * c : CH * (c + 1)], ps_cs[c])

            # ---- main matmul + epilogue ---------------------------------
            def epilogue(m, pys):
                yo = youtp.tile([P, O], BF16, tag="yo")
                for c in range(NCH):
                    sl = slice(CH * c, CH * (c + 1))
                    nc.scalar.mul(yo[:, sl], pys[c], acol[:, m : m + 1])
                tmp = epp.tile([P, O], BF16, tag="ep")
                nc.vector.tensor_scalar(
                    out=tmp, in0=cs_f, scalar1=ccol[:, m : m + 1],
                    scalar2=None, op0=Alu.mult,
                )
                nc.vector.tensor_tensor(out=yo, in0=yo, in1=tmp, op=Alu.subtract)
                nc.sync.dma_start(y[P * m : P * (m + 1), :], yo)

            def main_mm(m):
                if m == 0:
                    pys = pys0
                else:
                    pys = [
                        ps8.tile([P, CH], F32, tag=f"a{c}", name=f"py{m}_{c}")
                        for c in range(NCH)
                    ]
                    s, j = m // MPS, m % MPS
                    for k in range(KT):
                        lhs = xbig[s][:, CH * k + P * j : CH * k + P * (j + 1)]
                        for c in range(NCH):
                            nc.tensor.matmul(
                                pys[c], lhs, bw8s[k][:, CH * c : CH * (c + 1)],
                                start=(k == 0), stop=(k == KT - 1),
                            )
                epilogue(m, pys)

            # interleave remaining x slabs (ingest early; their stats
            # matmuls late so the PE never waits on them) between m-groups.
            for kp in range(KT // 2):
                x_ingest(1, kp)
            epilogue(0, pys0)
            main_mm(1)
            for g in range(NS):
                stats_mm(1, g)
            stats_fin(1)
            main_mm(2)
            main_mm(3)
            for kp in range(KT // 2):
                x_ingest(2, kp)
            main_mm(4)
            main_mm(5)
            for g in range(NS):
                stats_mm(2, g)
            stats_fin(2)
            main_mm(6)
            for kp in range(KT // 2):
                x_ingest(3, kp)
            main_mm(7)
            main_mm(8)
            main_mm(9)
            for g in range(NS):
                stats_mm(3, g)
            stats_fin(3)
            for m in range(10, MT):
                main_mm(m)

    nc.compile()
    return nc


_NC_CACHE = None


def _get_nc():
    global _NC_CACHE
    if _NC_CACHE is None:
        _NC_CACHE = build_nc()
    return _NC_CACHE


def _prep_in_maps(x, fweight):
    x2 = np.ascontiguousarray(x, dtype=np.float32).reshape(N_TOK, D)
    fwt = np.ascontiguousarray(np.asarray(fweight, dtype=np.float32).T)
    in_maps = []
    for c in range(N_CORES):
        xs = np.ascontiguousarray(x2[c * TOK : (c + 1) * TOK, :].T)
        in_maps.append({"xt": xs, "fwt": fwt})
    return in_maps


def run_spmd(x, fweight, **kw):
    nc = _get_nc()
    in_maps = _prep_in_maps(x, fweight)
    return run_bass_kernel_spmd(nc, in_maps, core_ids=list(range(N_CORES)), **kw)


def kernel(x, fweight):
    res = run_spmd(x, fweight)
    y = np.concatenate(
        [np.asarray(res.results[c]["y"], dtype=np.float32) for c in range(N_CORES)],
        axis=0,
    )
    return y.reshape(4, 4096, O)


if __name__ == "__main__":
    xx = np.random.randn(4, 4096, D).astype(np.float32)
    ww = np.random.uniform(-1 / np.sqrt(D), 1 / np.sqrt(D), (O, D)).astype(np.float32)
    out = kernel(xx, ww)
    print("out", out.shape, out.dtype, float(np.abs(out).mean()))
